# revision 5
# baseline (speedup 1.0000x reference)
"""GCN (2-layer GraphConv + edge scorer) on 8 Trainium2 NeuronCores — v5.

Strategy (dst-sharded, per-node edge slots, no per-edge scatter matmuls):
  - Nodes permuted by (lo, hi) src-half in-degree and dealt into
    8 cores x 49 blocks x 128 partitions so each block's nodes have
    near-equal degree; node p of a block owns partition p.
  - Edge slot (p, t) = t-th in-edge of node p, tiles padded to the block
    max degree. Two tile orderings: L1 uses block-major tiles (lo+hi runs
    adjacent) to keep one PSUM accumulator per block; L2/L3/scores use
    half-major streams (all lo tiles, then all hi tiles) so every
    dma_gather call is a full 8-tile (1024-row) call regardless of block
    boundaries (int16 gather indices force the half split of the 50176-row
    tables).
  - Layer 1 reads a host-staged, pre-gathered stream G1[p, slot, :] =
    (X * rsq_out)[src] with plain sequential DMA (no descriptor
    generation), sums slots per node with identity-lhsT matmuls
    accumulating in PSUM, then transposes the per-block aggregate and
    applies W1 (+b1, relu, * rsq_out) -> x1s rows (bf16) -> AllGather.
  - Layer 2 fetches x1s rows with dma_gather across 4 SWDGE queues; the
    lo pass parks per-block partial sums in SBUF, the hi pass combines,
    then the same transpose + W2 tail -> x2; per-node score halves
    s_src/s_dst via mult+reduce on the vector engine.
  - Scores: AllGather the [node, 64] s_src table, dma_gather per edge
    slot, sigmoid fused with the (s_dst + bp) per-partition bias.
Host does index preprocessing (degree sort, slot layout, staging G1) and
reassembles per-core score tiles into the original edge order.
"""
import os
import sys

_REPO = os.environ.get("TRN_RL_REPO", "/opt/trn_rl_repo")
if _REPO not in sys.path:
    sys.path.insert(0, _REPO)

import numpy as np
import ml_dtypes

import concourse.bass as bass
import concourse.bacc as bacc
import concourse.tile as tile
from concourse import mybir
from concourse.bass_utils import run_bass_kernel_spmd

P = 128
NCORES = 8
N_NODES = 50000
NPAD = 50176            # 8 * 49 * 128
BPC = NPAD // NCORES // P   # blocks per core = 49
NPC = NPAD // NCORES        # nodes per core = 6272
NLO = NPAD // 2             # 25088 rows in the lo half-table
LO_REAL = NLO - 1           # original ids 0..25086 are lo; id 50000 pads lo
IN_F = 256
HID = 256
OUT_F = 128

f32 = mybir.dt.float32
bf16 = mybir.dt.bfloat16
i16 = mybir.dt.int16
MAX_GT = 8   # dma_gather ucode limit: <=1024 indices per call
NQ = 4       # SWDGE queues

bfdt = ml_dtypes.bfloat16
AF = mybir.ActivationFunctionType


def _wrap_idx(idx_flat):
    """dma_gather index layout: idx k -> [k%16, k//16], replicated 8x."""
    n = idx_flat.shape[0]
    w = idx_flat.reshape(n // 16, 16).T
    return np.tile(w, (8, 1)).astype(np.int16)


def build_program(T_lo, T_hi):
    NB = len(T_lo)
    SL = int(sum(T_lo))          # lo-stream tiles
    SH = int(sum(T_hi))
    TE = SL + SH

    nc = bacc.Bacc("TRN2", target_bir_lowering=False, debug=False,
                   enable_asserts=True, num_devices=NCORES,
                   num_swdge_queues=NQ, dynamic_dma_scratch_size=65536)

    g1_d = nc.dram_tensor("g1", [P, TE, IN_F], bf16, kind="ExternalInput")
    src16_d = nc.dram_tensor("src16", [P, 8 * TE], i16, kind="ExternalInput")
    w1_d = nc.dram_tensor("w1c", [P, 2, HID], bf16, kind="ExternalInput")
    w2_d = nc.dram_tensor("w2c", [P, 2, OUT_F], bf16, kind="ExternalInput")
    ident_d = nc.dram_tensor("ident", [P, P], bf16, kind="ExternalInput")
    b1_d = nc.dram_tensor("b1r", [P, HID], f32, kind="ExternalInput")
    b2_d = nc.dram_tensor("b2r", [P, OUT_F], f32, kind="ExternalInput")
    wpt_d = nc.dram_tensor("wptr", [P, OUT_F], f32, kind="ExternalInput")
    wpb_d = nc.dram_tensor("wpbr", [P, OUT_F], f32, kind="ExternalInput")
    rsqi_d = nc.dram_tensor("rsqi", [P, NB], f32, kind="ExternalInput")
    rsqo_d = nc.dram_tensor("rsqo", [P, NB], f32, kind="ExternalInput")
    bp_d = nc.dram_tensor("bp", [P, 1], f32, kind="ExternalInput")
    scores_d = nc.dram_tensor("scores", [P, TE], f32, kind="ExternalOutput")

    qc = [0]

    def nextq():
        q = qc[0] % NQ
        qc[0] += 1
        return q

    # stream walk helper: tiles of the half-major stream with per-block runs
    def stream_runs(T_arr):
        """[(block, first_tile_in_stream, ntiles)] skipping empty blocks."""
        runs = []
        t0 = 0
        for b in range(NB):
            t = int(T_arr[b])
            if t:
                runs.append((b, t0, t))
            t0 += t
        return runs

    with tile.TileContext(nc) as tc:
        with (
            tc.tile_pool(name="cons", bufs=1) as cons,
            tc.tile_pool(name="sb", bufs=2) as sb,
            tc.tile_pool(name="ps", bufs=2, space="PSUM") as ps,
            tc.tile_pool(name="dram", bufs=1, space="DRAM") as dr,
        ):
            src16 = cons.tile([P, 8 * TE], i16, name="src16")
            w1c = cons.tile([P, 2, HID], bf16, name="w1c")
            w2c = cons.tile([P, 2, OUT_F], bf16, name="w2c")
            ident = cons.tile([P, P], bf16, name="ident")
            b1r = cons.tile([P, HID], f32, name="b1r")
            b2r = cons.tile([P, OUT_F], f32, name="b2r")
            wptr = cons.tile([P, OUT_F], f32, name="wptr")
            wpbr = cons.tile([P, OUT_F], f32, name="wpbr")
            rsqi = cons.tile([P, NB], f32, name="rsqi")
            rsqo = cons.tile([P, NB], f32, name="rsqo")
            bp_sb = cons.tile([P, 1], f32, name="bp_sb")
            sdst_all = cons.tile([P, NB], f32, name="sdst_all")
            acc_lo = cons.tile([P, NB, HID], f32, name="acc_lo")
            for s_t, d_t in [(src16, src16_d), (w1c, w1_d), (w2c, w2_d),
                             (ident, ident_d), (b1r, b1_d), (b2r, b2_d),
                             (wptr, wpt_d), (wpbr, wpb_d), (rsqi, rsqi_d),
                             (rsqo, rsqo_d), (bp_sb, bp_d)]:
                nc.sync.dma_start(s_t[:], d_t[:])

            x1s_slice = dr.tile([NPC, HID], bf16, name="x1s_slice")
            x1s_full = dr.tile([NPAD, HID], bf16, name="x1s_full")
            s_slice = dr.tile([NPC, 64], f32, name="s_slice")
            s_full = dr.tile([NPAD, 64], f32, name="s_full")

            def finish_layer(b, acc_bf, wc, brow, d_out):
                """acc (bf16 [P, d_in]) -> transpose -> @W -> z*rsqi+b,relu."""
                d_in = acc_bf.shape[-1]
                nch = d_in // P
                accT = sb.tile([P, nch, P], bf16, tag="accT", name="accT",
                               bufs=3)
                for c in range(nch):
                    tp = ps.tile([P, P], bf16, tag="tp", name="tp", bufs=2)
                    nc.tensor.transpose(tp[:], acc_bf[:, c * P:(c + 1) * P],
                                        ident[:])
                    nc.vector.tensor_copy(accT[:, c, :], tp[:])
                z_ps = ps.tile([P, d_out], f32, tag="z", name="z_ps")
                for c in range(nch):
                    nc.tensor.matmul(out=z_ps[:], lhsT=accT[:, c, :],
                                     rhs=wc[:, c, :],
                                     start=(c == 0), stop=(c == nch - 1))
                t1 = sb.tile([P, d_out], f32, tag="t1", name="t1", bufs=3)
                nc.scalar.activation(t1[:], z_ps[:], AF.Copy,
                                     scale=rsqi[:, b:b + 1])
                t2 = sb.tile([P, d_out], f32, tag="t2", name="t2", bufs=3)
                nc.vector.tensor_tensor(out=t2[:], in0=t1[:], in1=brow[:],
                                        op=mybir.AluOpType.add)
                xf = sb.tile([P, d_out], f32, tag="xf", name="xf", bufs=3)
                nc.scalar.activation(xf[:], t2[:], AF.Relu)
                return xf

            # ---------------- layer 1 (block-major G1 stream) ----------------
            gt0 = 0
            for b in range(NB):
                T = int(T_lo[b]) + int(T_hi[b])
                acc_bf = sb.tile([P, IN_F], bf16, tag="acc_bf", name="acc_bf",
                                 bufs=3)
                if T == 0:
                    nc.vector.memset(acc_bf[:], 0.0)
                else:
                    acc_ps = ps.tile([P, IN_F], f32, tag="acc", name="acc_ps")
                    done = 0
                    while done < T:
                        n = min(MAX_GT, T - done)
                        g = sb.tile([P, MAX_GT, IN_F], bf16, tag="g1t",
                                    name="g1t", bufs=4)
                        t0 = gt0 + done
                        nc.sync.dma_start(g[:, 0:n, :], g1_d[:, t0:t0 + n, :])
                        for i in range(n):
                            nc.tensor.matmul(
                                out=acc_ps[:], lhsT=ident[:], rhs=g[:, i, :],
                                start=(done + i == 0),
                                stop=(done + i == T - 1))
                        done += n
                    nc.scalar.activation(acc_bf[:], acc_ps[:], AF.Copy)
                xf = finish_layer(b, acc_bf, w1c, b1r, HID)
                x1s = sb.tile([P, HID], bf16, tag="x1s", name="x1s", bufs=3)
                nc.scalar.activation(x1s[:], xf[:], AF.Copy,
                                     scale=rsqo[:, b:b + 1])
                nc.sync.dma_start(x1s_slice[b * P:(b + 1) * P, :], x1s[:])
                gt0 += T

            nc.gpsimd.collective_compute(
                "AllGather", mybir.AluOpType.bypass,
                replica_groups=[list(range(NCORES))],
                ins=[x1s_slice.opt()], outs=[x1s_full.opt()])

            # ---------------- layer 2 (half-major gather streams) ------------
            def l2_pass(T_arr, stream_base, table, on_block_done):
                """One half pass: full 8-tile gather calls over the stream;
                per-block PSUM accumulate within each block's run."""
                runs = stream_runs(T_arr)
                if not runs:
                    return
                total = sum(r[2] for r in runs)
                # walk 8-tile chunks of the stream; runs are contiguous
                ri = 0          # current run index
                off_in_run = 0
                acc_ps = None
                t = 0
                while t < total:
                    n = min(MAX_GT, total - t)
                    g = sb.tile([P, MAX_GT, HID], bf16, tag="g2t",
                                name="g2t", bufs=6)
                    st0 = stream_base + t
                    nc.gpsimd.dma_gather(
                        g[:, 0:n, :], table,
                        src16[:, 8 * st0:8 * (st0 + n)],
                        P * n, P * n, HID, queue_num=nextq())
                    for i in range(n):
                        b, rt0, rn = runs[ri]
                        if off_in_run == 0:
                            acc_ps = ps.tile([P, HID], f32, tag="acc",
                                             name="acc_ps")
                        nc.tensor.matmul(
                            out=acc_ps[:], lhsT=ident[:], rhs=g[:, i, :],
                            start=(off_in_run == 0),
                            stop=(off_in_run == rn - 1))
                        off_in_run += 1
                        if off_in_run == rn:
                            on_block_done(b, acc_ps)
                            ri += 1
                            off_in_run = 0
                    t += n

            def lo_done(b, acc_ps):
                nc.scalar.activation(acc_lo[:, b, :], acc_ps[:], AF.Copy)

            l2_pass(T_lo, 0, x1s_full[0:NLO, :], lo_done)
            for b in range(NB):
                if T_lo[b] == 0:
                    nc.vector.memset(acc_lo[:, b, :], 0.0)

            done_hi = [False] * NB

            def l2_tail(b, acc_bf):
                x2 = finish_layer(b, acc_bf, w2c, b2r, OUT_F)
                scr = sb.tile([P, OUT_F], f32, tag="scr", name="scr", bufs=3)
                ssrc = sb.tile([P, 1], f32, tag="ssrc", name="ssrc", bufs=3)
                nc.vector.tensor_tensor(out=scr[:], in0=x2[:], in1=wptr[:],
                                        op=mybir.AluOpType.mult)
                nc.vector.tensor_reduce(out=ssrc[:], in_=scr[:],
                                        op=mybir.AluOpType.add,
                                        axis=mybir.AxisListType.X)
                scr2 = sb.tile([P, OUT_F], f32, tag="scr2", name="scr2",
                               bufs=3)
                sdst = sb.tile([P, 1], f32, tag="sdst", name="sdst", bufs=3)
                nc.vector.tensor_tensor(out=scr2[:], in0=x2[:], in1=wpbr[:],
                                        op=mybir.AluOpType.mult)
                nc.vector.tensor_reduce(out=sdst[:], in_=scr2[:],
                                        op=mybir.AluOpType.add,
                                        axis=mybir.AxisListType.X)
                nc.vector.tensor_scalar(out=sdst_all[:, b:b + 1], in0=sdst[:],
                                        scalar1=bp_sb[:, 0:1], scalar2=None,
                                        op0=mybir.AluOpType.add)
                sblk = sb.tile([P, 64], f32, tag="sblk", name="sblk", bufs=3)
                nc.vector.tensor_copy(sblk[:],
                                      ssrc[:, 0:1].to_broadcast([P, 64]))
                nc.sync.dma_start(s_slice[b * P:(b + 1) * P, :], sblk[:])

            def hi_done(b, acc_ps):
                done_hi[b] = True
                acc_bf = sb.tile([P, HID], bf16, tag="acc_bf", name="acc_bf",
                                 bufs=3)
                nc.vector.tensor_tensor(out=acc_bf[:], in0=acc_ps[:],
                                        in1=acc_lo[:, b, :],
                                        op=mybir.AluOpType.add)
                l2_tail(b, acc_bf)

            l2_pass(T_hi, SL, x1s_full[NLO:NPAD, :], hi_done)
            for b in range(NB):
                if not done_hi[b]:
                    acc_bf = sb.tile([P, HID], bf16, tag="acc_bf",
                                     name="acc_bf", bufs=3)
                    nc.vector.tensor_copy(acc_bf[:], acc_lo[:, b, :])
                    l2_tail(b, acc_bf)

            nc.gpsimd.collective_compute(
                "AllGather", mybir.AluOpType.bypass,
                replica_groups=[list(range(NCORES))],
                ins=[s_slice.opt()], outs=[s_full.opt()])

            # ---------------- edge scores (half-major streams) ---------------
            def score_pass(T_arr, stream_base, table):
                runs = stream_runs(T_arr)
                if not runs:
                    return
                total = sum(r[2] for r in runs)
                # per-tile block id for activation segmentation
                tile_blk = []
                for b, rt0, rn in runs:
                    tile_blk += [b] * rn
                t = 0
                while t < total:
                    n = min(MAX_GT, total - t)
                    st0 = stream_base + t
                    gA = sb.tile([P, MAX_GT, 64], f32, tag="gA",
                                 name="gA", bufs=6)
                    nc.gpsimd.dma_gather(
                        gA[:, 0:n, :], table,
                        src16[:, 8 * st0:8 * (st0 + n)],
                        P * n, P * n, 64, queue_num=nextq())
                    sc = sb.tile([P, MAX_GT], f32, tag="sc", name="sc",
                                 bufs=6)
                    i = 0
                    while i < n:
                        b = tile_blk[t + i]
                        j = i
                        while j < n and tile_blk[t + j] == b:
                            j += 1
                        nc.scalar.activation(
                            sc[:, i:j], gA[:, i:j, 0], AF.Sigmoid,
                            bias=sdst_all[:, b:b + 1])
                        i = j
                    nc.sync.dma_start(scores_d[:, st0:st0 + n], sc[:, 0:n])
                    t += n

            score_pass(T_lo, 0, s_full[0:NLO, :])
            score_pass(T_hi, SL, s_full[NLO:NPAD, :])

    nc.compile()
    return nc


def preprocess(features, src, dst, W1, b1, W2, b2, Wp, bp):
    E = src.shape[0]
    src = src.astype(np.int64)
    dst = dst.astype(np.int64)

    deg_out = np.bincount(src, minlength=NPAD).astype(np.float64)
    deg_in = np.bincount(dst, minlength=NPAD).astype(np.float64)
    rsq_out = (1.0 / np.sqrt(np.clip(deg_out, 1.0, None))).astype(np.float32)
    rsq_in = (1.0 / np.sqrt(np.clip(deg_in, 1.0, None))).astype(np.float32)
    rsq_out[N_NODES:] = 0.0   # kill pad-node rows in the x1s table

    halfm = (src >= LO_REAL).astype(np.int64)   # 1 = src in hi table
    dlo = np.bincount(dst[halfm == 0], minlength=NPAD)
    dhi = np.bincount(dst[halfm == 1], minlength=NPAD)

    lo_ids = np.concatenate([np.arange(LO_REAL), [N_NODES]])
    hi_ids = np.concatenate([np.arange(LO_REAL, N_NODES),
                             np.arange(N_NODES + 1, NPAD)])

    pos = np.empty(NPAD, np.int64)
    T_lo = np.zeros(BPC, np.int64)
    T_hi = np.zeros(BPC, np.int64)
    for half, ids, base_core in ((0, lo_ids, 0), (1, hi_ids, 4)):
        order = np.lexsort((dhi[ids], dlo[ids]))[::-1]
        ids_sorted = ids[order]
        blocks = ids_sorted.reshape(4 * BPC, P)
        kl = dlo[blocks].max(axis=1)
        kh = dhi[blocks].max(axis=1)
        for i in range(4 * BPC):
            core = base_core + i % 4
            slot = i // 4
            pos[blocks[i]] = core * NPC + slot * P + np.arange(P)
            T_lo[slot] = max(T_lo[slot], kl[i])
            T_hi[slot] = max(T_hi[slot], kh[i])
    SL = int(T_lo.sum())
    TE = SL + int(T_hi.sum())

    # ord1 (L1/G1): block-major tiles, lo run then hi run within a block
    gt0_1 = np.zeros(BPC, np.int64)
    gt0_1[1:] = np.cumsum(T_lo + T_hi)[:-1]
    # ord2 (L2/L3/scores): all lo tiles by block, then all hi tiles
    lo_t0 = np.zeros(BPC, np.int64)
    lo_t0[1:] = np.cumsum(T_lo)[:-1]
    hi_t0 = np.zeros(BPC, np.int64)
    hi_t0[1:] = np.cumsum(T_hi)[:-1]
    hi_t0 += SL

    # rank of each edge within (dst-node, half)
    pdst = pos[dst]
    key = pdst * 2 + halfm
    order = np.argsort(key, kind="stable")
    ks = key[order]
    starts = np.searchsorted(ks, np.arange(2 * NPAD + 1))
    rank = np.arange(E) - starts[ks]

    core_e = pdst[order] // NPC
    b_e = (pdst[order] % NPC) // P
    p_e = pdst[order] % P
    h_e = halfm[order]
    t1_e = np.where(h_e == 1, T_lo[b_e] + rank, rank)
    slot1_e = (gt0_1[b_e] + t1_e) * P + p_e
    t2_e = np.where(h_e == 1, hi_t0[b_e] + rank, lo_t0[b_e] + rank)
    slot2_e = t2_e * P + p_e
    srcpos_e = pos[src[order]]

    zlo = pos[N_NODES]          # lo-half zero row (pad node id 50000)
    zhi = pos[N_NODES + 1]      # hi-half zero row

    # ord1 slots for G1 staging
    srcpos1 = np.full((NCORES, TE * P), zlo, np.int64)
    srcpos1[core_e, slot1_e] = srcpos_e
    # ord2 slots for gathers + score reassembly
    pad2 = np.where(np.repeat(np.arange(TE) >= SL, P), zhi, zlo)
    srcpos2 = np.tile(pad2, (NCORES, 1))
    slot_orig = np.full((NCORES, TE * P), -1, np.int64)
    srcpos2[core_e, slot2_e] = srcpos_e
    slot_orig[core_e, slot2_e] = order

    # permuted, scaled node feature table
    T1p = np.zeros((NPAD, IN_F), np.float32)
    T1p[pos[:N_NODES]] = features * rsq_out[:N_NODES, None]
    T1p_bf = T1p.astype(bfdt)

    inv = np.empty(NPAD, np.int64)
    inv[pos] = np.arange(NPAD)
    rsqi_cols = rsq_in[inv].reshape(NCORES, BPC, P).transpose(0, 2, 1)
    rsqo_cols = rsq_out[inv].reshape(NCORES, BPC, P).transpose(0, 2, 1)

    # w1c[p, c, j] = W1[c*128+p, j]
    w1c = np.stack([W1[0:P, :], W1[P:2 * P, :]], axis=0).transpose(1, 0, 2)
    w2c = np.stack([W2[0:P, :], W2[P:2 * P, :]], axis=0).transpose(1, 0, 2)
    ident = np.eye(P, dtype=np.float32)
    b1r = np.broadcast_to(b1.astype(np.float32)[None, :], (P, HID)).copy()
    b2r = np.broadcast_to(b2.astype(np.float32)[None, :], (P, OUT_F)).copy()
    wptr = np.broadcast_to(Wp[:OUT_F, 0].astype(np.float32)[None, :],
                           (P, OUT_F)).copy()
    wpbr = np.broadcast_to(Wp[OUT_F:, 0].astype(np.float32)[None, :],
                           (P, OUT_F)).copy()
    bp_t = np.full((P, 1), np.float32(bp[0]))

    in_maps = []
    for core in range(NCORES):
        g1 = np.ascontiguousarray(
            T1p_bf[srcpos1[core]].reshape(TE, P, IN_F).transpose(1, 0, 2))
        slots = srcpos2[core]
        s16 = np.where(slots < NLO, slots, slots - NLO)
        in_maps.append(dict(
            g1=g1, src16=_wrap_idx(s16),
            w1c=w1c.astype(bfdt), w2c=w2c.astype(bfdt),
            ident=ident.astype(bfdt), b1r=b1r, b2r=b2r,
            wptr=wptr, wpbr=wpbr,
            rsqi=np.ascontiguousarray(rsqi_cols[core]),
            rsqo=np.ascontiguousarray(rsqo_cols[core]),
            bp=bp_t,
        ))
    return in_maps, slot_orig, T_lo, T_hi, E


_CACHE = {}


def _get_program(T_lo, T_hi):
    key = (tuple(T_lo), tuple(T_hi))
    if key not in _CACHE:
        _CACHE[key] = build_program(T_lo, T_hi)
    return _CACHE[key]


def kernel(features, src, dst, edge_type, W1, b1, W2, b2, Wp, bp,
           _trace=False, _tmpdir=None):
    features = np.asarray(features, np.float32)
    src_i = np.asarray(src, np.int32)
    dst_i = np.asarray(dst, np.int32)
    in_maps, slot_orig, T_lo, T_hi, E = preprocess(
        features, src_i, dst_i, np.asarray(W1), np.asarray(b1),
        np.asarray(W2), np.asarray(b2), np.asarray(Wp), np.asarray(bp))
    nc = _get_program(T_lo, T_hi)
    res = run_bass_kernel_spmd(nc, in_maps, core_ids=list(range(NCORES)),
                               trace=_trace, tmpdir=_tmpdir)
    out = np.zeros(E, np.float32)
    for core in range(NCORES):
        sc = res.results[core]["scores"]        # [P, TE]
        flat = sc.T.reshape(-1)                 # slot q = tile*128+p
        so = slot_orig[core]
        m = so >= 0
        out[so[m]] = flat[m]
    if _trace:
        kernel._last_results = res
    return out


# revision 6
# speedup vs baseline: 1.0002x; 1.0002x over previous
"""GCN (2-layer GraphConv + edge scorer) on 8 Trainium2 NeuronCores — v5.

Strategy (dst-sharded, per-node edge slots, no per-edge scatter matmuls):
  - Nodes permuted by (lo, hi) src-half in-degree and dealt into
    8 cores x 49 blocks x 128 partitions so each block's nodes have
    near-equal degree; node p of a block owns partition p.
  - Edge slot (p, t) = t-th in-edge of node p, tiles padded to the block
    max degree. Two tile orderings: L1 uses block-major tiles (lo+hi runs
    adjacent) to keep one PSUM accumulator per block; L2/L3/scores use
    half-major streams (all lo tiles, then all hi tiles) so every
    dma_gather call is a full 8-tile (1024-row) call regardless of block
    boundaries (int16 gather indices force the half split of the 50176-row
    tables).
  - Layer 1 reads a host-staged, pre-gathered stream G1[p, slot, :] =
    (X * rsq_out)[src] with plain sequential DMA (no descriptor
    generation), sums slots per node with identity-lhsT matmuls
    accumulating in PSUM, then transposes the per-block aggregate and
    applies W1 (+b1, relu, * rsq_out) -> x1s rows (bf16) -> AllGather.
  - Layer 2 fetches x1s rows with dma_gather across 4 SWDGE queues; the
    lo pass parks per-block partial sums in SBUF, the hi pass combines,
    then the same transpose + W2 tail -> x2; per-node score halves
    s_src/s_dst via mult+reduce on the vector engine.
  - Scores: AllGather the [node, 64] s_src table, dma_gather per edge
    slot, sigmoid fused with the (s_dst + bp) per-partition bias.
Host does index preprocessing (degree sort, slot layout, staging G1) and
reassembles per-core score tiles into the original edge order.
"""
import os
import sys

_REPO = os.environ.get("TRN_RL_REPO", "/opt/trn_rl_repo")
if _REPO not in sys.path:
    sys.path.insert(0, _REPO)

import numpy as np
import ml_dtypes

import concourse.bass as bass
import concourse.bacc as bacc
import concourse.tile as tile
from concourse import mybir
from concourse.bass_utils import run_bass_kernel_spmd

P = 128
NCORES = 8
N_NODES = 50000
NPAD = 50176            # 8 * 49 * 128
BPC = NPAD // NCORES // P   # blocks per core = 49
NPC = NPAD // NCORES        # nodes per core = 6272
NLO = NPAD // 2             # 25088 rows in the lo half-table
LO_REAL = NLO - 1           # original ids 0..25086 are lo; id 50000 pads lo
IN_F = 256
HID = 256
OUT_F = 128

f32 = mybir.dt.float32
bf16 = mybir.dt.bfloat16
i16 = mybir.dt.int16
MAX_GT = 8   # dma_gather ucode limit: <=1024 indices per call
NQ = 4       # SWDGE queues

bfdt = ml_dtypes.bfloat16
AF = mybir.ActivationFunctionType


def _wrap_idx(idx_flat):
    """dma_gather index layout: idx k -> [k%16, k//16], replicated 8x."""
    n = idx_flat.shape[0]
    w = idx_flat.reshape(n // 16, 16).T
    return np.tile(w, (8, 1)).astype(np.int16)


def build_program(T_lo, T_hi):
    NB = len(T_lo)
    SL = int(sum(T_lo))          # lo-stream tiles
    SH = int(sum(T_hi))
    TE = SL + SH

    nc = bacc.Bacc("TRN2", target_bir_lowering=False, debug=False,
                   enable_asserts=True, num_devices=NCORES,
                   num_swdge_queues=NQ, dynamic_dma_scratch_size=98304)

    g1_d = nc.dram_tensor("g1", [P, TE, IN_F], bf16, kind="ExternalInput")
    src16_d = nc.dram_tensor("src16", [P, 8 * TE], i16, kind="ExternalInput")
    w1_d = nc.dram_tensor("w1c", [P, 2, HID], bf16, kind="ExternalInput")
    w2_d = nc.dram_tensor("w2c", [P, 2, OUT_F], bf16, kind="ExternalInput")
    ident_d = nc.dram_tensor("ident", [P, P], bf16, kind="ExternalInput")
    b1_d = nc.dram_tensor("b1r", [P, HID], f32, kind="ExternalInput")
    b2_d = nc.dram_tensor("b2r", [P, OUT_F], f32, kind="ExternalInput")
    wpt_d = nc.dram_tensor("wptr", [P, OUT_F], f32, kind="ExternalInput")
    wpb_d = nc.dram_tensor("wpbr", [P, OUT_F], f32, kind="ExternalInput")
    rsqi_d = nc.dram_tensor("rsqi", [P, NB], f32, kind="ExternalInput")
    rsqo_d = nc.dram_tensor("rsqo", [P, NB], f32, kind="ExternalInput")
    bp_d = nc.dram_tensor("bp", [P, 1], f32, kind="ExternalInput")
    scores_d = nc.dram_tensor("scores", [P, TE], f32, kind="ExternalOutput")

    qc = [0]

    def nextq():
        q = qc[0] % NQ
        qc[0] += 1
        return q

    # stream walk helper: tiles of the half-major stream with per-block runs
    def stream_runs(T_arr):
        """[(block, first_tile_in_stream, ntiles)] skipping empty blocks."""
        runs = []
        t0 = 0
        for b in range(NB):
            t = int(T_arr[b])
            if t:
                runs.append((b, t0, t))
            t0 += t
        return runs

    with tile.TileContext(nc) as tc:
        with (
            tc.tile_pool(name="cons", bufs=1) as cons,
            tc.tile_pool(name="sb", bufs=2) as sb,
            tc.tile_pool(name="ps", bufs=2, space="PSUM") as ps,
            tc.tile_pool(name="dram", bufs=1, space="DRAM") as dr,
        ):
            src16 = cons.tile([P, 8 * TE], i16, name="src16")
            w1c = cons.tile([P, 2, HID], bf16, name="w1c")
            w2c = cons.tile([P, 2, OUT_F], bf16, name="w2c")
            ident = cons.tile([P, P], bf16, name="ident")
            b1r = cons.tile([P, HID], f32, name="b1r")
            b2r = cons.tile([P, OUT_F], f32, name="b2r")
            wptr = cons.tile([P, OUT_F], f32, name="wptr")
            wpbr = cons.tile([P, OUT_F], f32, name="wpbr")
            rsqi = cons.tile([P, NB], f32, name="rsqi")
            rsqo = cons.tile([P, NB], f32, name="rsqo")
            bp_sb = cons.tile([P, 1], f32, name="bp_sb")
            sdst_all = cons.tile([P, NB], f32, name="sdst_all")
            acc_lo = cons.tile([P, NB, HID], bf16, name="acc_lo")
            for s_t, d_t in [(src16, src16_d), (w1c, w1_d), (w2c, w2_d),
                             (ident, ident_d), (b1r, b1_d), (b2r, b2_d),
                             (wptr, wpt_d), (wpbr, wpb_d), (rsqi, rsqi_d),
                             (rsqo, rsqo_d), (bp_sb, bp_d)]:
                nc.sync.dma_start(s_t[:], d_t[:])

            x1s_slice = dr.tile([NPC, HID], bf16, name="x1s_slice")
            x1s_full = dr.tile([NPAD, HID], bf16, name="x1s_full")
            s_slice = dr.tile([NPC, 64], f32, name="s_slice")
            s_full = dr.tile([NPAD, 64], f32, name="s_full")

            def finish_layer(b, acc_bf, wc, brow, d_out):
                """acc (bf16 [P, d_in]) -> transpose -> @W -> z*rsqi+b,relu."""
                d_in = acc_bf.shape[-1]
                nch = d_in // P
                accT = sb.tile([P, nch, P], bf16, tag="accT", name="accT",
                               bufs=3)
                for c in range(nch):
                    tp = ps.tile([P, P], bf16, tag="tp", name="tp", bufs=2)
                    nc.tensor.transpose(tp[:], acc_bf[:, c * P:(c + 1) * P],
                                        ident[:])
                    nc.vector.tensor_copy(accT[:, c, :], tp[:])
                z_ps = ps.tile([P, d_out], f32, tag="z", name="z_ps")
                for c in range(nch):
                    nc.tensor.matmul(out=z_ps[:], lhsT=accT[:, c, :],
                                     rhs=wc[:, c, :],
                                     start=(c == 0), stop=(c == nch - 1))
                t1 = sb.tile([P, d_out], f32, tag="t1", name="t1", bufs=3)
                nc.scalar.activation(t1[:], z_ps[:], AF.Copy,
                                     scale=rsqi[:, b:b + 1])
                t2 = sb.tile([P, d_out], f32, tag="t2", name="t2", bufs=3)
                nc.vector.tensor_tensor(out=t2[:], in0=t1[:], in1=brow[:],
                                        op=mybir.AluOpType.add)
                xf = sb.tile([P, d_out], f32, tag="xf", name="xf", bufs=3)
                nc.scalar.activation(xf[:], t2[:], AF.Relu)
                return xf

            # ---------------- layer 1 (block-major G1 stream) ----------------
            gt0 = 0
            for b in range(NB):
                T = int(T_lo[b]) + int(T_hi[b])
                acc_bf = sb.tile([P, IN_F], bf16, tag="acc_bf", name="acc_bf",
                                 bufs=3)
                if T == 0:
                    nc.vector.memset(acc_bf[:], 0.0)
                else:
                    acc_ps = ps.tile([P, IN_F], f32, tag="acc", name="acc_ps")
                    done = 0
                    while done < T:
                        n = min(MAX_GT, T - done)
                        g = sb.tile([P, MAX_GT, IN_F], bf16, tag="g1t",
                                    name="g1t", bufs=4)
                        t0 = gt0 + done
                        nc.sync.dma_start(g[:, 0:n, :], g1_d[:, t0:t0 + n, :])
                        for i in range(n):
                            nc.tensor.matmul(
                                out=acc_ps[:], lhsT=ident[:], rhs=g[:, i, :],
                                start=(done + i == 0),
                                stop=(done + i == T - 1))
                        done += n
                    nc.scalar.activation(acc_bf[:], acc_ps[:], AF.Copy)
                xf = finish_layer(b, acc_bf, w1c, b1r, HID)
                x1s = sb.tile([P, HID], bf16, tag="x1s", name="x1s", bufs=3)
                nc.scalar.activation(x1s[:], xf[:], AF.Copy,
                                     scale=rsqo[:, b:b + 1])
                nc.sync.dma_start(x1s_slice[b * P:(b + 1) * P, :], x1s[:])
                gt0 += T

            nc.gpsimd.collective_compute(
                "AllGather", mybir.AluOpType.bypass,
                replica_groups=[list(range(NCORES))],
                ins=[x1s_slice.opt()], outs=[x1s_full.opt()])

            # ---------------- layer 2 (half-major gather streams) ------------
            def l2_pass(T_arr, stream_base, table, on_block_done):
                """One half pass: full 8-tile gather calls over the stream;
                per-block PSUM accumulate within each block's run."""
                runs = stream_runs(T_arr)
                if not runs:
                    return
                total = sum(r[2] for r in runs)
                # walk 8-tile chunks of the stream; runs are contiguous
                ri = 0          # current run index
                off_in_run = 0
                acc_ps = None
                t = 0
                while t < total:
                    n = min(MAX_GT, total - t)
                    g = sb.tile([P, MAX_GT, HID], bf16, tag="g2t",
                                name="g2t", bufs=8)
                    st0 = stream_base + t
                    nc.gpsimd.dma_gather(
                        g[:, 0:n, :], table,
                        src16[:, 8 * st0:8 * (st0 + n)],
                        P * n, P * n, HID, queue_num=nextq())
                    for i in range(n):
                        b, rt0, rn = runs[ri]
                        if off_in_run == 0:
                            acc_ps = ps.tile([P, HID], f32, tag="acc",
                                             name="acc_ps")
                        nc.tensor.matmul(
                            out=acc_ps[:], lhsT=ident[:], rhs=g[:, i, :],
                            start=(off_in_run == 0),
                            stop=(off_in_run == rn - 1))
                        off_in_run += 1
                        if off_in_run == rn:
                            on_block_done(b, acc_ps)
                            ri += 1
                            off_in_run = 0
                    t += n

            def lo_done(b, acc_ps):
                nc.scalar.activation(acc_lo[:, b, :], acc_ps[:], AF.Copy)

            l2_pass(T_lo, 0, x1s_full[0:NLO, :], lo_done)
            for b in range(NB):
                if T_lo[b] == 0:
                    nc.vector.memset(acc_lo[:, b, :], 0.0)

            done_hi = [False] * NB

            def l2_tail(b, acc_bf):
                x2 = finish_layer(b, acc_bf, w2c, b2r, OUT_F)
                scr = sb.tile([P, OUT_F], f32, tag="scr", name="scr", bufs=3)
                ssrc = sb.tile([P, 1], f32, tag="ssrc", name="ssrc", bufs=3)
                nc.vector.tensor_tensor(out=scr[:], in0=x2[:], in1=wptr[:],
                                        op=mybir.AluOpType.mult)
                nc.vector.tensor_reduce(out=ssrc[:], in_=scr[:],
                                        op=mybir.AluOpType.add,
                                        axis=mybir.AxisListType.X)
                scr2 = sb.tile([P, OUT_F], f32, tag="scr2", name="scr2",
                               bufs=3)
                sdst = sb.tile([P, 1], f32, tag="sdst", name="sdst", bufs=3)
                nc.vector.tensor_tensor(out=scr2[:], in0=x2[:], in1=wpbr[:],
                                        op=mybir.AluOpType.mult)
                nc.vector.tensor_reduce(out=sdst[:], in_=scr2[:],
                                        op=mybir.AluOpType.add,
                                        axis=mybir.AxisListType.X)
                nc.vector.tensor_scalar(out=sdst_all[:, b:b + 1], in0=sdst[:],
                                        scalar1=bp_sb[:, 0:1], scalar2=None,
                                        op0=mybir.AluOpType.add)
                sblk = sb.tile([P, 64], f32, tag="sblk", name="sblk", bufs=3)
                nc.vector.tensor_copy(sblk[:],
                                      ssrc[:, 0:1].to_broadcast([P, 64]))
                nc.sync.dma_start(s_slice[b * P:(b + 1) * P, :], sblk[:])

            def hi_done(b, acc_ps):
                done_hi[b] = True
                acc_bf = sb.tile([P, HID], bf16, tag="acc_bf", name="acc_bf",
                                 bufs=3)
                nc.vector.tensor_tensor(out=acc_bf[:], in0=acc_ps[:],
                                        in1=acc_lo[:, b, :],
                                        op=mybir.AluOpType.add)
                l2_tail(b, acc_bf)

            l2_pass(T_hi, SL, x1s_full[NLO:NPAD, :], hi_done)
            for b in range(NB):
                if not done_hi[b]:
                    acc_bf = sb.tile([P, HID], bf16, tag="acc_bf",
                                     name="acc_bf", bufs=3)
                    nc.vector.tensor_copy(acc_bf[:], acc_lo[:, b, :])
                    l2_tail(b, acc_bf)

            nc.gpsimd.collective_compute(
                "AllGather", mybir.AluOpType.bypass,
                replica_groups=[list(range(NCORES))],
                ins=[s_slice.opt()], outs=[s_full.opt()])

            # ---------------- edge scores (half-major streams) ---------------
            def score_pass(T_arr, stream_base, table):
                runs = stream_runs(T_arr)
                if not runs:
                    return
                total = sum(r[2] for r in runs)
                # per-tile block id for activation segmentation
                tile_blk = []
                for b, rt0, rn in runs:
                    tile_blk += [b] * rn
                t = 0
                while t < total:
                    n = min(MAX_GT, total - t)
                    st0 = stream_base + t
                    gA = sb.tile([P, MAX_GT, 64], f32, tag="gA",
                                 name="gA", bufs=8)
                    nc.gpsimd.dma_gather(
                        gA[:, 0:n, :], table,
                        src16[:, 8 * st0:8 * (st0 + n)],
                        P * n, P * n, 64, queue_num=nextq())
                    sc = sb.tile([P, MAX_GT], f32, tag="sc", name="sc",
                                 bufs=6)
                    i = 0
                    while i < n:
                        b = tile_blk[t + i]
                        j = i
                        while j < n and tile_blk[t + j] == b:
                            j += 1
                        nc.scalar.activation(
                            sc[:, i:j], gA[:, i:j, 0], AF.Sigmoid,
                            bias=sdst_all[:, b:b + 1])
                        i = j
                    nc.sync.dma_start(scores_d[:, st0:st0 + n], sc[:, 0:n])
                    t += n

            score_pass(T_lo, 0, s_full[0:NLO, :])
            score_pass(T_hi, SL, s_full[NLO:NPAD, :])

    nc.compile()
    return nc


def preprocess(features, src, dst, W1, b1, W2, b2, Wp, bp):
    E = src.shape[0]
    src = src.astype(np.int64)
    dst = dst.astype(np.int64)

    deg_out = np.bincount(src, minlength=NPAD).astype(np.float64)
    deg_in = np.bincount(dst, minlength=NPAD).astype(np.float64)
    rsq_out = (1.0 / np.sqrt(np.clip(deg_out, 1.0, None))).astype(np.float32)
    rsq_in = (1.0 / np.sqrt(np.clip(deg_in, 1.0, None))).astype(np.float32)
    rsq_out[N_NODES:] = 0.0   # kill pad-node rows in the x1s table

    halfm = (src >= LO_REAL).astype(np.int64)   # 1 = src in hi table
    dlo = np.bincount(dst[halfm == 0], minlength=NPAD)
    dhi = np.bincount(dst[halfm == 1], minlength=NPAD)

    lo_ids = np.concatenate([np.arange(LO_REAL), [N_NODES]])
    hi_ids = np.concatenate([np.arange(LO_REAL, N_NODES),
                             np.arange(N_NODES + 1, NPAD)])

    pos = np.empty(NPAD, np.int64)
    T_lo = np.zeros(BPC, np.int64)
    T_hi = np.zeros(BPC, np.int64)
    for half, ids, base_core in ((0, lo_ids, 0), (1, hi_ids, 4)):
        order = np.lexsort((dhi[ids], dlo[ids]))[::-1]
        ids_sorted = ids[order]
        blocks = ids_sorted.reshape(4 * BPC, P)
        kl = dlo[blocks].max(axis=1)
        kh = dhi[blocks].max(axis=1)
        for i in range(4 * BPC):
            core = base_core + i % 4
            slot = i // 4
            pos[blocks[i]] = core * NPC + slot * P + np.arange(P)
            T_lo[slot] = max(T_lo[slot], kl[i])
            T_hi[slot] = max(T_hi[slot], kh[i])
    SL = int(T_lo.sum())
    TE = SL + int(T_hi.sum())

    # ord1 (L1/G1): block-major tiles, lo run then hi run within a block
    gt0_1 = np.zeros(BPC, np.int64)
    gt0_1[1:] = np.cumsum(T_lo + T_hi)[:-1]
    # ord2 (L2/L3/scores): all lo tiles by block, then all hi tiles
    lo_t0 = np.zeros(BPC, np.int64)
    lo_t0[1:] = np.cumsum(T_lo)[:-1]
    hi_t0 = np.zeros(BPC, np.int64)
    hi_t0[1:] = np.cumsum(T_hi)[:-1]
    hi_t0 += SL

    # rank of each edge within (dst-node, half), ordered by src position
    # ascending so a tile's 128 reads cluster in a narrow table band
    pdst = pos[dst]
    key = pdst * 2 + halfm
    order = np.lexsort((pos[src], key))
    ks = key[order]
    starts = np.searchsorted(ks, np.arange(2 * NPAD + 1))
    rank = np.arange(E) - starts[ks]

    core_e = pdst[order] // NPC
    b_e = (pdst[order] % NPC) // P
    p_e = pdst[order] % P
    h_e = halfm[order]
    t1_e = np.where(h_e == 1, T_lo[b_e] + rank, rank)
    slot1_e = (gt0_1[b_e] + t1_e) * P + p_e
    t2_e = np.where(h_e == 1, hi_t0[b_e] + rank, lo_t0[b_e] + rank)
    slot2_e = t2_e * P + p_e
    srcpos_e = pos[src[order]]

    zlo = pos[N_NODES]          # lo-half zero row (pad node id 50000)
    zhi = pos[N_NODES + 1]      # hi-half zero row

    # ord1 slots for G1 staging
    srcpos1 = np.full((NCORES, TE * P), zlo, np.int64)
    srcpos1[core_e, slot1_e] = srcpos_e
    # ord2 slots for gathers + score reassembly
    pad2 = np.where(np.repeat(np.arange(TE) >= SL, P), zhi, zlo)
    srcpos2 = np.tile(pad2, (NCORES, 1))
    slot_orig = np.full((NCORES, TE * P), -1, np.int64)
    srcpos2[core_e, slot2_e] = srcpos_e
    slot_orig[core_e, slot2_e] = order

    # permuted, scaled node feature table
    T1p = np.zeros((NPAD, IN_F), np.float32)
    T1p[pos[:N_NODES]] = features * rsq_out[:N_NODES, None]
    T1p_bf = T1p.astype(bfdt)

    inv = np.empty(NPAD, np.int64)
    inv[pos] = np.arange(NPAD)
    rsqi_cols = rsq_in[inv].reshape(NCORES, BPC, P).transpose(0, 2, 1)
    rsqo_cols = rsq_out[inv].reshape(NCORES, BPC, P).transpose(0, 2, 1)

    # w1c[p, c, j] = W1[c*128+p, j]
    w1c = np.stack([W1[0:P, :], W1[P:2 * P, :]], axis=0).transpose(1, 0, 2)
    w2c = np.stack([W2[0:P, :], W2[P:2 * P, :]], axis=0).transpose(1, 0, 2)
    ident = np.eye(P, dtype=np.float32)
    b1r = np.broadcast_to(b1.astype(np.float32)[None, :], (P, HID)).copy()
    b2r = np.broadcast_to(b2.astype(np.float32)[None, :], (P, OUT_F)).copy()
    wptr = np.broadcast_to(Wp[:OUT_F, 0].astype(np.float32)[None, :],
                           (P, OUT_F)).copy()
    wpbr = np.broadcast_to(Wp[OUT_F:, 0].astype(np.float32)[None, :],
                           (P, OUT_F)).copy()
    bp_t = np.full((P, 1), np.float32(bp[0]))

    in_maps = []
    for core in range(NCORES):
        g1 = np.ascontiguousarray(
            T1p_bf[srcpos1[core]].reshape(TE, P, IN_F).transpose(1, 0, 2))
        slots = srcpos2[core]
        s16 = np.where(slots < NLO, slots, slots - NLO)
        in_maps.append(dict(
            g1=g1, src16=_wrap_idx(s16),
            w1c=w1c.astype(bfdt), w2c=w2c.astype(bfdt),
            ident=ident.astype(bfdt), b1r=b1r, b2r=b2r,
            wptr=wptr, wpbr=wpbr,
            rsqi=np.ascontiguousarray(rsqi_cols[core]),
            rsqo=np.ascontiguousarray(rsqo_cols[core]),
            bp=bp_t,
        ))
    return in_maps, slot_orig, T_lo, T_hi, E


_CACHE = {}


def _get_program(T_lo, T_hi):
    key = (tuple(T_lo), tuple(T_hi))
    if key not in _CACHE:
        _CACHE[key] = build_program(T_lo, T_hi)
    return _CACHE[key]


def kernel(features, src, dst, edge_type, W1, b1, W2, b2, Wp, bp,
           _trace=False, _tmpdir=None):
    features = np.asarray(features, np.float32)
    src_i = np.asarray(src, np.int32)
    dst_i = np.asarray(dst, np.int32)
    in_maps, slot_orig, T_lo, T_hi, E = preprocess(
        features, src_i, dst_i, np.asarray(W1), np.asarray(b1),
        np.asarray(W2), np.asarray(b2), np.asarray(Wp), np.asarray(bp))
    nc = _get_program(T_lo, T_hi)
    res = run_bass_kernel_spmd(nc, in_maps, core_ids=list(range(NCORES)),
                               trace=_trace, tmpdir=_tmpdir)
    out = np.zeros(E, np.float32)
    for core in range(NCORES):
        sc = res.results[core]["scores"]        # [P, TE]
        flat = sc.T.reshape(-1)                 # slot q = tile*128+p
        so = slot_orig[core]
        m = so >= 0
        out[so[m]] = flat[m]
    if _trace:
        kernel._last_results = res
    return out


# revision 7
# speedup vs baseline: 1.0702x; 1.0700x over previous
"""GCN (2-layer GraphConv + edge scorer) on 8 Trainium2 NeuronCores — v5.

Strategy (dst-sharded, per-node edge slots, no per-edge scatter matmuls):
  - Nodes permuted by (lo, hi) src-half in-degree and dealt into
    8 cores x 49 blocks x 128 partitions so each block's nodes have
    near-equal degree; node p of a block owns partition p.
  - Edge slot (p, t) = t-th in-edge of node p, tiles padded to the block
    max degree. Two tile orderings: L1 uses block-major tiles (lo+hi runs
    adjacent) to keep one PSUM accumulator per block; L2/L3/scores use
    half-major streams (all lo tiles, then all hi tiles) so every
    dma_gather call is a full 8-tile (1024-row) call regardless of block
    boundaries (int16 gather indices force the half split of the 50176-row
    tables).
  - Layer 1 reads a host-staged, pre-gathered stream G1[p, slot, :] =
    (X * rsq_out)[src] with plain sequential DMA (no descriptor
    generation), sums slots per node with identity-lhsT matmuls
    accumulating in PSUM, then transposes the per-block aggregate and
    applies W1 (+b1, relu, * rsq_out) -> x1s rows (bf16) -> AllGather.
  - Layer 2 fetches x1s rows with dma_gather across 4 SWDGE queues; the
    lo pass parks per-block partial sums in SBUF, the hi pass combines,
    then the same transpose + W2 tail -> x2; per-node score halves
    s_src/s_dst via mult+reduce on the vector engine.
  - Scores: AllGather the [node, 64] s_src table, dma_gather per edge
    slot, sigmoid fused with the (s_dst + bp) per-partition bias.
Host does index preprocessing (degree sort, slot layout, staging G1) and
reassembles per-core score tiles into the original edge order.
"""
import os
import sys

_REPO = os.environ.get("TRN_RL_REPO", "/opt/trn_rl_repo")
if _REPO not in sys.path:
    sys.path.insert(0, _REPO)

import numpy as np
import ml_dtypes

import concourse.bass as bass
import concourse.bacc as bacc
import concourse.tile as tile
from concourse import mybir
from concourse.bass_utils import run_bass_kernel_spmd

P = 128
NCORES = 8
N_NODES = 50000
NPAD = 50176            # 8 * 49 * 128
BPC = NPAD // NCORES // P   # blocks per core = 49
NPC = NPAD // NCORES        # nodes per core = 6272
NLO = NPAD // 2             # 25088 rows in the lo half-table
LO_REAL = NLO - 1           # original ids 0..25086 are lo; id 50000 pads lo
IN_F = 256
HID = 256
OUT_F = 128

f32 = mybir.dt.float32
bf16 = mybir.dt.bfloat16
i16 = mybir.dt.int16
MAX_GT = 8   # dma_gather ucode limit: <=1024 indices per call
NQ = 4       # SWDGE queues

bfdt = ml_dtypes.bfloat16
AF = mybir.ActivationFunctionType


def _wrap_idx(idx_flat):
    """dma_gather index layout: idx k -> [k%16, k//16], replicated 8x."""
    n = idx_flat.shape[0]
    w = idx_flat.reshape(n // 16, 16).T
    return np.tile(w, (8, 1)).astype(np.int16)


def build_program(T_lo, T_hi):
    NB = len(T_lo)
    SL = int(sum(T_lo))          # lo-stream tiles
    SH = int(sum(T_hi))
    TE = SL + SH

    nc = bacc.Bacc("TRN2", target_bir_lowering=False, debug=False,
                   enable_asserts=True, num_devices=NCORES,
                   num_swdge_queues=NQ, dynamic_dma_scratch_size=98304)

    g1_d = nc.dram_tensor("g1", [P, TE, IN_F], bf16, kind="ExternalInput")
    src16_d = nc.dram_tensor("src16", [P, 8 * TE], i16, kind="ExternalInput")
    w1_d = nc.dram_tensor("w1c", [P, 2, HID], bf16, kind="ExternalInput")
    w2_d = nc.dram_tensor("w2c", [P, 2, OUT_F], bf16, kind="ExternalInput")
    ident_d = nc.dram_tensor("ident", [P, P], bf16, kind="ExternalInput")
    b1_d = nc.dram_tensor("b1r", [P, HID], f32, kind="ExternalInput")
    b2_d = nc.dram_tensor("b2r", [P, OUT_F], f32, kind="ExternalInput")
    wpt_d = nc.dram_tensor("wptr", [P, OUT_F], f32, kind="ExternalInput")
    wpb_d = nc.dram_tensor("wpbr", [P, OUT_F], f32, kind="ExternalInput")
    rsqi_d = nc.dram_tensor("rsqi", [P, NB], f32, kind="ExternalInput")
    rsqo_d = nc.dram_tensor("rsqo", [P, NB], f32, kind="ExternalInput")
    bp_d = nc.dram_tensor("bp", [P, 1], f32, kind="ExternalInput")
    scores_d = nc.dram_tensor("scores", [P, TE], f32, kind="ExternalOutput")

    qc = [0]

    def nextq():
        q = qc[0] % NQ
        qc[0] += 1
        return q

    # stream walk helper: tiles of the half-major stream with per-block runs
    def stream_runs(T_arr):
        """[(block, first_tile_in_stream, ntiles)] skipping empty blocks."""
        runs = []
        t0 = 0
        for b in range(NB):
            t = int(T_arr[b])
            if t:
                runs.append((b, t0, t))
            t0 += t
        return runs

    with tile.TileContext(nc) as tc:
        with (
            tc.tile_pool(name="cons", bufs=1) as cons,
            tc.tile_pool(name="sb", bufs=2) as sb,
            tc.tile_pool(name="ps", bufs=2, space="PSUM") as ps,
            tc.tile_pool(name="dram", bufs=1, space="DRAM") as dr,
        ):
            src16 = cons.tile([P, 8 * TE], i16, name="src16")
            w1c = cons.tile([P, 2, HID], bf16, name="w1c")
            w2c = cons.tile([P, 2, OUT_F], bf16, name="w2c")
            ident = cons.tile([P, P], bf16, name="ident")
            b1r = cons.tile([P, HID], f32, name="b1r")
            b2r = cons.tile([P, OUT_F], f32, name="b2r")
            wptr = cons.tile([P, OUT_F], f32, name="wptr")
            wpbr = cons.tile([P, OUT_F], f32, name="wpbr")
            rsqi = cons.tile([P, NB], f32, name="rsqi")
            rsqo = cons.tile([P, NB], f32, name="rsqo")
            bp_sb = cons.tile([P, 1], f32, name="bp_sb")
            sdst_all = cons.tile([P, NB], f32, name="sdst_all")
            acc_lo = cons.tile([P, NB, HID], bf16, name="acc_lo")
            for s_t, d_t in [(src16, src16_d), (w1c, w1_d), (w2c, w2_d),
                             (ident, ident_d), (b1r, b1_d), (b2r, b2_d),
                             (wptr, wpt_d), (wpbr, wpb_d), (rsqi, rsqi_d),
                             (rsqo, rsqo_d), (bp_sb, bp_d)]:
                nc.sync.dma_start(s_t[:], d_t[:])

            x1s_slice = dr.tile([NPC, HID], bf16, name="x1s_slice")
            s_slice = dr.tile([NPC, 64], f32, name="s_slice")
            x1s_full = nc.dram_tensor("x1s_full", [NPAD, HID], bf16,
                                      kind="Internal", addr_space="Shared")
            s_full = nc.dram_tensor("s_full", [NPAD, 64], f32,
                                    kind="Internal", addr_space="Shared")

            def finish_layer(b, acc_bf, wc, brow, d_out):
                """acc (bf16 [P, d_in]) -> transpose -> @W -> z*rsqi+b,relu."""
                d_in = acc_bf.shape[-1]
                nch = d_in // P
                accT = sb.tile([P, nch, P], bf16, tag="accT", name="accT",
                               bufs=3)
                for c in range(nch):
                    tp = ps.tile([P, P], bf16, tag="tp", name="tp", bufs=2)
                    nc.tensor.transpose(tp[:], acc_bf[:, c * P:(c + 1) * P],
                                        ident[:])
                    nc.vector.tensor_copy(accT[:, c, :], tp[:])
                z_ps = ps.tile([P, d_out], f32, tag="z", name="z_ps")
                for c in range(nch):
                    nc.tensor.matmul(out=z_ps[:], lhsT=accT[:, c, :],
                                     rhs=wc[:, c, :],
                                     start=(c == 0), stop=(c == nch - 1))
                t1 = sb.tile([P, d_out], f32, tag="t1", name="t1", bufs=3)
                nc.scalar.activation(t1[:], z_ps[:], AF.Copy,
                                     scale=rsqi[:, b:b + 1])
                t2 = sb.tile([P, d_out], f32, tag="t2", name="t2", bufs=3)
                nc.vector.tensor_tensor(out=t2[:], in0=t1[:], in1=brow[:],
                                        op=mybir.AluOpType.add)
                xf = sb.tile([P, d_out], f32, tag="xf", name="xf", bufs=3)
                nc.scalar.activation(xf[:], t2[:], AF.Relu)
                return xf

            # ---------------- layer 1 (block-major G1 stream) ----------------
            gt0 = 0
            for b in range(NB):
                T = int(T_lo[b]) + int(T_hi[b])
                acc_bf = sb.tile([P, IN_F], bf16, tag="acc_bf", name="acc_bf",
                                 bufs=3)
                if T == 0:
                    nc.vector.memset(acc_bf[:], 0.0)
                else:
                    acc_ps = ps.tile([P, IN_F], f32, tag="acc", name="acc_ps")
                    done = 0
                    while done < T:
                        n = min(MAX_GT, T - done)
                        g = sb.tile([P, MAX_GT, IN_F], bf16, tag="g1t",
                                    name="g1t", bufs=4)
                        t0 = gt0 + done
                        nc.sync.dma_start(g[:, 0:n, :], g1_d[:, t0:t0 + n, :])
                        for i in range(n):
                            nc.tensor.matmul(
                                out=acc_ps[:], lhsT=ident[:], rhs=g[:, i, :],
                                start=(done + i == 0),
                                stop=(done + i == T - 1))
                        done += n
                    nc.scalar.activation(acc_bf[:], acc_ps[:], AF.Copy)
                xf = finish_layer(b, acc_bf, w1c, b1r, HID)
                x1s = sb.tile([P, HID], bf16, tag="x1s", name="x1s", bufs=3)
                nc.scalar.activation(x1s[:], xf[:], AF.Copy,
                                     scale=rsqo[:, b:b + 1])
                nc.sync.dma_start(x1s_slice[b * P:(b + 1) * P, :], x1s[:])
                gt0 += T

            nc.gpsimd.collective_compute(
                "AllGather", mybir.AluOpType.bypass,
                replica_groups=[list(range(NCORES))],
                ins=[x1s_slice.opt()], outs=[x1s_full[:]])

            # ---------------- layer 2 (half-major gather streams) ------------
            def l2_pass(T_arr, stream_base, table, on_block_done):
                """One half pass: full 8-tile gather calls over the stream;
                per-block PSUM accumulate within each block's run."""
                runs = stream_runs(T_arr)
                if not runs:
                    return
                total = sum(r[2] for r in runs)
                # walk 8-tile chunks of the stream; runs are contiguous
                ri = 0          # current run index
                off_in_run = 0
                acc_ps = None
                t = 0
                while t < total:
                    n = min(MAX_GT, total - t)
                    g = sb.tile([P, MAX_GT, HID], bf16, tag="g2t",
                                name="g2t", bufs=8)
                    st0 = stream_base + t
                    nc.gpsimd.dma_gather(
                        g[:, 0:n, :], table,
                        src16[:, 8 * st0:8 * (st0 + n)],
                        P * n, P * n, HID, queue_num=nextq())
                    for i in range(n):
                        b, rt0, rn = runs[ri]
                        if off_in_run == 0:
                            acc_ps = ps.tile([P, HID], f32, tag="acc",
                                             name="acc_ps")
                        nc.tensor.matmul(
                            out=acc_ps[:], lhsT=ident[:], rhs=g[:, i, :],
                            start=(off_in_run == 0),
                            stop=(off_in_run == rn - 1))
                        off_in_run += 1
                        if off_in_run == rn:
                            on_block_done(b, acc_ps)
                            ri += 1
                            off_in_run = 0
                    t += n

            def lo_done(b, acc_ps):
                nc.scalar.activation(acc_lo[:, b, :], acc_ps[:], AF.Copy)

            l2_pass(T_lo, 0, x1s_full[0:NLO, :], lo_done)
            for b in range(NB):
                if T_lo[b] == 0:
                    nc.vector.memset(acc_lo[:, b, :], 0.0)

            done_hi = [False] * NB

            def l2_tail(b, acc_bf):
                x2 = finish_layer(b, acc_bf, w2c, b2r, OUT_F)
                scr = sb.tile([P, OUT_F], f32, tag="scr", name="scr", bufs=3)
                ssrc = sb.tile([P, 1], f32, tag="ssrc", name="ssrc", bufs=3)
                nc.vector.tensor_tensor(out=scr[:], in0=x2[:], in1=wptr[:],
                                        op=mybir.AluOpType.mult)
                nc.vector.tensor_reduce(out=ssrc[:], in_=scr[:],
                                        op=mybir.AluOpType.add,
                                        axis=mybir.AxisListType.X)
                scr2 = sb.tile([P, OUT_F], f32, tag="scr2", name="scr2",
                               bufs=3)
                sdst = sb.tile([P, 1], f32, tag="sdst", name="sdst", bufs=3)
                nc.vector.tensor_tensor(out=scr2[:], in0=x2[:], in1=wpbr[:],
                                        op=mybir.AluOpType.mult)
                nc.vector.tensor_reduce(out=sdst[:], in_=scr2[:],
                                        op=mybir.AluOpType.add,
                                        axis=mybir.AxisListType.X)
                nc.vector.tensor_scalar(out=sdst_all[:, b:b + 1], in0=sdst[:],
                                        scalar1=bp_sb[:, 0:1], scalar2=None,
                                        op0=mybir.AluOpType.add)
                sblk = sb.tile([P, 64], f32, tag="sblk", name="sblk", bufs=3)
                nc.vector.tensor_copy(sblk[:],
                                      ssrc[:, 0:1].to_broadcast([P, 64]))
                nc.sync.dma_start(s_slice[b * P:(b + 1) * P, :], sblk[:])

            def hi_done(b, acc_ps):
                done_hi[b] = True
                acc_bf = sb.tile([P, HID], bf16, tag="acc_bf", name="acc_bf",
                                 bufs=3)
                nc.vector.tensor_tensor(out=acc_bf[:], in0=acc_ps[:],
                                        in1=acc_lo[:, b, :],
                                        op=mybir.AluOpType.add)
                l2_tail(b, acc_bf)

            l2_pass(T_hi, SL, x1s_full[NLO:NPAD, :], hi_done)
            for b in range(NB):
                if not done_hi[b]:
                    acc_bf = sb.tile([P, HID], bf16, tag="acc_bf",
                                     name="acc_bf", bufs=3)
                    nc.vector.tensor_copy(acc_bf[:], acc_lo[:, b, :])
                    l2_tail(b, acc_bf)

            nc.gpsimd.collective_compute(
                "AllGather", mybir.AluOpType.bypass,
                replica_groups=[list(range(NCORES))],
                ins=[s_slice.opt()], outs=[s_full[:]])

            # ---------------- edge scores (half-major streams) ---------------
            def score_pass(T_arr, stream_base, table):
                runs = stream_runs(T_arr)
                if not runs:
                    return
                total = sum(r[2] for r in runs)
                # per-tile block id for activation segmentation
                tile_blk = []
                for b, rt0, rn in runs:
                    tile_blk += [b] * rn
                t = 0
                while t < total:
                    n = min(MAX_GT, total - t)
                    st0 = stream_base + t
                    gA = sb.tile([P, MAX_GT, 64], f32, tag="gA",
                                 name="gA", bufs=8)
                    nc.gpsimd.dma_gather(
                        gA[:, 0:n, :], table,
                        src16[:, 8 * st0:8 * (st0 + n)],
                        P * n, P * n, 64, queue_num=nextq())
                    sc = sb.tile([P, MAX_GT], f32, tag="sc", name="sc",
                                 bufs=6)
                    i = 0
                    while i < n:
                        b = tile_blk[t + i]
                        j = i
                        while j < n and tile_blk[t + j] == b:
                            j += 1
                        nc.scalar.activation(
                            sc[:, i:j], gA[:, i:j, 0], AF.Sigmoid,
                            bias=sdst_all[:, b:b + 1])
                        i = j
                    nc.sync.dma_start(scores_d[:, st0:st0 + n], sc[:, 0:n])
                    t += n

            score_pass(T_lo, 0, s_full[0:NLO, :])
            score_pass(T_hi, SL, s_full[NLO:NPAD, :])

    nc.compile()
    return nc


def preprocess(features, src, dst, W1, b1, W2, b2, Wp, bp):
    E = src.shape[0]
    src = src.astype(np.int64)
    dst = dst.astype(np.int64)

    deg_out = np.bincount(src, minlength=NPAD).astype(np.float64)
    deg_in = np.bincount(dst, minlength=NPAD).astype(np.float64)
    rsq_out = (1.0 / np.sqrt(np.clip(deg_out, 1.0, None))).astype(np.float32)
    rsq_in = (1.0 / np.sqrt(np.clip(deg_in, 1.0, None))).astype(np.float32)
    rsq_out[N_NODES:] = 0.0   # kill pad-node rows in the x1s table

    halfm = (src >= LO_REAL).astype(np.int64)   # 1 = src in hi table
    dlo = np.bincount(dst[halfm == 0], minlength=NPAD)
    dhi = np.bincount(dst[halfm == 1], minlength=NPAD)

    lo_ids = np.concatenate([np.arange(LO_REAL), [N_NODES]])
    hi_ids = np.concatenate([np.arange(LO_REAL, N_NODES),
                             np.arange(N_NODES + 1, NPAD)])

    pos = np.empty(NPAD, np.int64)
    T_lo = np.zeros(BPC, np.int64)
    T_hi = np.zeros(BPC, np.int64)
    for half, ids, base_core in ((0, lo_ids, 0), (1, hi_ids, 4)):
        order = np.lexsort((dhi[ids], dlo[ids]))[::-1]
        ids_sorted = ids[order]
        blocks = ids_sorted.reshape(4 * BPC, P)
        kl = dlo[blocks].max(axis=1)
        kh = dhi[blocks].max(axis=1)
        for i in range(4 * BPC):
            core = base_core + i % 4
            slot = i // 4
            pos[blocks[i]] = core * NPC + slot * P + np.arange(P)
            T_lo[slot] = max(T_lo[slot], kl[i])
            T_hi[slot] = max(T_hi[slot], kh[i])
    SL = int(T_lo.sum())
    TE = SL + int(T_hi.sum())

    # ord1 (L1/G1): block-major tiles, lo run then hi run within a block
    gt0_1 = np.zeros(BPC, np.int64)
    gt0_1[1:] = np.cumsum(T_lo + T_hi)[:-1]
    # ord2 (L2/L3/scores): all lo tiles by block, then all hi tiles
    lo_t0 = np.zeros(BPC, np.int64)
    lo_t0[1:] = np.cumsum(T_lo)[:-1]
    hi_t0 = np.zeros(BPC, np.int64)
    hi_t0[1:] = np.cumsum(T_hi)[:-1]
    hi_t0 += SL

    # rank of each edge within (dst-node, half), ordered by src position
    # ascending so a tile's 128 reads cluster in a narrow table band
    pdst = pos[dst]
    key = pdst * 2 + halfm
    order = np.lexsort((pos[src], key))
    ks = key[order]
    starts = np.searchsorted(ks, np.arange(2 * NPAD + 1))
    rank = np.arange(E) - starts[ks]

    core_e = pdst[order] // NPC
    b_e = (pdst[order] % NPC) // P
    p_e = pdst[order] % P
    h_e = halfm[order]
    t1_e = np.where(h_e == 1, T_lo[b_e] + rank, rank)
    slot1_e = (gt0_1[b_e] + t1_e) * P + p_e
    t2_e = np.where(h_e == 1, hi_t0[b_e] + rank, lo_t0[b_e] + rank)
    slot2_e = t2_e * P + p_e
    srcpos_e = pos[src[order]]

    zlo = pos[N_NODES]          # lo-half zero row (pad node id 50000)
    zhi = pos[N_NODES + 1]      # hi-half zero row

    # ord1 slots for G1 staging
    srcpos1 = np.full((NCORES, TE * P), zlo, np.int64)
    srcpos1[core_e, slot1_e] = srcpos_e
    # ord2 slots for gathers + score reassembly
    pad2 = np.where(np.repeat(np.arange(TE) >= SL, P), zhi, zlo)
    srcpos2 = np.tile(pad2, (NCORES, 1))
    slot_orig = np.full((NCORES, TE * P), -1, np.int64)
    srcpos2[core_e, slot2_e] = srcpos_e
    slot_orig[core_e, slot2_e] = order

    # permuted, scaled node feature table
    T1p = np.zeros((NPAD, IN_F), np.float32)
    T1p[pos[:N_NODES]] = features * rsq_out[:N_NODES, None]
    T1p_bf = T1p.astype(bfdt)

    inv = np.empty(NPAD, np.int64)
    inv[pos] = np.arange(NPAD)
    rsqi_cols = rsq_in[inv].reshape(NCORES, BPC, P).transpose(0, 2, 1)
    rsqo_cols = rsq_out[inv].reshape(NCORES, BPC, P).transpose(0, 2, 1)

    # w1c[p, c, j] = W1[c*128+p, j]
    w1c = np.stack([W1[0:P, :], W1[P:2 * P, :]], axis=0).transpose(1, 0, 2)
    w2c = np.stack([W2[0:P, :], W2[P:2 * P, :]], axis=0).transpose(1, 0, 2)
    ident = np.eye(P, dtype=np.float32)
    b1r = np.broadcast_to(b1.astype(np.float32)[None, :], (P, HID)).copy()
    b2r = np.broadcast_to(b2.astype(np.float32)[None, :], (P, OUT_F)).copy()
    wptr = np.broadcast_to(Wp[:OUT_F, 0].astype(np.float32)[None, :],
                           (P, OUT_F)).copy()
    wpbr = np.broadcast_to(Wp[OUT_F:, 0].astype(np.float32)[None, :],
                           (P, OUT_F)).copy()
    bp_t = np.full((P, 1), np.float32(bp[0]))

    in_maps = []
    for core in range(NCORES):
        g1 = np.ascontiguousarray(
            T1p_bf[srcpos1[core]].reshape(TE, P, IN_F).transpose(1, 0, 2))
        slots = srcpos2[core]
        s16 = np.where(slots < NLO, slots, slots - NLO)
        in_maps.append(dict(
            g1=g1, src16=_wrap_idx(s16),
            w1c=w1c.astype(bfdt), w2c=w2c.astype(bfdt),
            ident=ident.astype(bfdt), b1r=b1r, b2r=b2r,
            wptr=wptr, wpbr=wpbr,
            rsqi=np.ascontiguousarray(rsqi_cols[core]),
            rsqo=np.ascontiguousarray(rsqo_cols[core]),
            bp=bp_t,
        ))
    return in_maps, slot_orig, T_lo, T_hi, E


_CACHE = {}


def _get_program(T_lo, T_hi):
    key = (tuple(T_lo), tuple(T_hi))
    if key not in _CACHE:
        _CACHE[key] = build_program(T_lo, T_hi)
    return _CACHE[key]


def kernel(features, src, dst, edge_type, W1, b1, W2, b2, Wp, bp,
           _trace=False, _tmpdir=None):
    features = np.asarray(features, np.float32)
    src_i = np.asarray(src, np.int32)
    dst_i = np.asarray(dst, np.int32)
    in_maps, slot_orig, T_lo, T_hi, E = preprocess(
        features, src_i, dst_i, np.asarray(W1), np.asarray(b1),
        np.asarray(W2), np.asarray(b2), np.asarray(Wp), np.asarray(bp))
    nc = _get_program(T_lo, T_hi)
    res = run_bass_kernel_spmd(nc, in_maps, core_ids=list(range(NCORES)),
                               trace=_trace, tmpdir=_tmpdir)
    out = np.zeros(E, np.float32)
    for core in range(NCORES):
        sc = res.results[core]["scores"]        # [P, TE]
        flat = sc.T.reshape(-1)                 # slot q = tile*128+p
        so = slot_orig[core]
        m = so >= 0
        out[so[m]] = flat[m]
    if _trace:
        kernel._last_results = res
    return out
